# revision 1
# baseline (speedup 1.0000x reference)
"""ExtAttention Trainium2 kernel (8 NeuronCores, SPMD).

Sharding: 8 cores = 4 batches x 2 query-row halves. Each core handles
batch b = core//2 and query rows [ih*1024, ih*1024+1024) with ih = core%2.
Softmax is over the key axis j (free dim), so row-sharding needs no
collectives; each core reads exactly its slice of the dominant `indicator`
tensor (42 MB/core) once.

Per-core dataflow (n=2048 keys, I=1024 query rows, H=4 heads, DH=32):
  - qkv projection on PE (q only for the local row half; scale folded into w_q)
  - per 32-row i-chunk and 512-col j-tile: one PSUM tile [(h,i32)=128, 512]
    accumulates three matmuls: sim (block-diag q stationary, K=(h,dh)=128),
    bias over indicator channels 0..3 (K=(c,i32)=128, sparse w_ind stationary),
    bias over channel 4 (K=32).
  - ACT exp PSUM->SBUF with accum_out giving row sums for free.
  - PE transpose (128x128 chunks) -> PSUM, DVE copy -> SBUF gives E^T.
  - av matmul: lhsT = v^T[j128,(h,d)], rhs = E^T[j128,(h,i32)], accumulated
    over all 16 j-chunks into one PSUM tile [(h,d),(h,i32)]; diagonal head
    blocks are the per-head attention outputs.
  - row-sum reciprocal is moved to the free dim with a DVE 32x32 block
    transpose; extraction of diag blocks fuses the 1/sum scaling (DVE STT).
  - output projection (w_out^T stationary) + bias, DMA out (256, 1024).
"""

import os
import sys

import numpy as np

for _p in ("/opt/trn_rl_repo", "/root/.axon_site/_ro/trn_rl_repo"):
    if os.path.isdir(_p) and _p not in sys.path:
        sys.path.insert(0, _p)

B, DIM, N, C, H, DH = 4, 256, 2048, 5, 4, 32
HID = H * DH            # 128
NCORES = 8
I = N // 2              # 1024 query rows per core
NIC = I // 32           # 32 i-chunks
NJT = N // 512          # 4 j-tiles
NJC = N // 128          # 16 j-chunks

_PROG = None
LAST_EXEC_NS = None
LAST_RESULTS = None


def _build_program(repeat=1):
    from contextlib import ExitStack

    import concourse.mybir as mybir
    import concourse.tile as tile
    from concourse import bacc
    from concourse.masks import make_identity

    f32 = mybir.dt.float32
    Alu = mybir.AluOpType
    Act = mybir.ActivationFunctionType

    nc = bacc.Bacc("TRN2", target_bir_lowering=False, debug=False,
                   num_devices=NCORES)

    x_d = nc.dram_tensor("x", [DIM, N], f32, kind="ExternalInput").ap()
    xq_d = nc.dram_tensor("xq", [DIM, I], f32, kind="ExternalInput").ap()
    ind_d = nc.dram_tensor("ind", [NIC, NJT, C * 32, 512], f32,
                           kind="ExternalInput").ap()
    wqkvT_d = nc.dram_tensor("wqkvT", [128, 2, 3 * HID], f32,
                             kind="ExternalInput").ap()
    s0_d = nc.dram_tensor("s0", [128, 128], f32, kind="ExternalInput").ap()
    s1_d = nc.dram_tensor("s1", [32, 128], f32, kind="ExternalInput").ap()
    woutT_d = nc.dram_tensor("woutT", [128, 2, 128], f32,
                             kind="ExternalInput").ap()
    bout_d = nc.dram_tensor("bout", [128, 2], f32, kind="ExternalInput").ap()
    out_d = nc.dram_tensor("out", [DIM, I], f32, kind="ExternalOutput").ap()

    with tile.TileContext(nc) as tc, ExitStack() as ctx:
        const = ctx.enter_context(tc.tile_pool(name="const", bufs=1))
        big = ctx.enter_context(tc.tile_pool(name="big", bufs=1))
        indp = ctx.enter_context(tc.tile_pool(name="indp", bufs=3))
        ep = ctx.enter_context(tc.tile_pool(name="ep", bufs=3))
        etp = ctx.enter_context(tc.tile_pool(name="etp", bufs=3))
        smallp = ctx.enter_context(tc.tile_pool(name="smallp", bufs=3))
        ps_mm = ctx.enter_context(tc.tile_pool(name="ps_mm", bufs=4, space="PSUM"))
        ps_t = ctx.enter_context(tc.tile_pool(name="ps_t", bufs=2, space="PSUM"))
        ps_av = ctx.enter_context(tc.tile_pool(name="ps_av", bufs=2, space="PSUM"))

        for _rep in range(repeat):
            # ---- constants ----
            wqkvT = const.tile([128, 2, 3 * HID], f32, tag="wqkvT")
            nc.sync.dma_start(wqkvT[:], wqkvT_d)
            s0 = const.tile([128, 128], f32, tag="s0")
            nc.sync.dma_start(s0[:], s0_d)
            s1 = const.tile([32, 128], f32, tag="s1")
            nc.sync.dma_start(s1[:], s1_d)
            woutT = const.tile([128, 2, 128], f32, tag="woutT")
            nc.sync.dma_start(woutT[:], woutT_d)
            bout = const.tile([128, 2], f32, tag="bout")
            nc.sync.dma_start(bout[:], bout_d)
            ident = const.tile([128, 128], f32, tag="ident")
            make_identity(nc, ident[:])

            # ---- load x ----
            x_sb = big.tile([128, 2, N], f32, tag="x_sb")
            xq_sb = big.tile([128, 2, I], f32, tag="xq_sb")
            for kc in range(2):
                nc.sync.dma_start(x_sb[:, kc, :], x_d[kc * 128:(kc + 1) * 128, :])
                nc.sync.dma_start(xq_sb[:, kc, :], xq_d[kc * 128:(kc + 1) * 128, :])

            # ---- qkv projection ----
            q_sb = big.tile([128, I], f32, tag="q_sb")     # [(h,d), i_local]  (scale folded)
            k_sb = big.tile([128, N], f32, tag="k_sb")     # [(h,d), j]
            v_sb = big.tile([128, N], f32, tag="v_sb")     # [(h,d), j]
            vT_sb = big.tile([128, NJC, 128], f32, tag="vT_sb")  # [j128, jc, (h,d)]

            for nt in range(I // 512):
                ps = ps_mm.tile([128, 512], f32, tag="mm")
                for kc in range(2):
                    nc.tensor.matmul(ps[:], wqkvT[:, kc, 0:128],
                                     xq_sb[:, kc, nt * 512:(nt + 1) * 512],
                                     start=(kc == 0), stop=(kc == 1))
                nc.scalar.copy(q_sb[:, nt * 512:(nt + 1) * 512], ps[:])
            for dst, lo in ((k_sb, 128), (v_sb, 256)):
                for nt in range(N // 512):
                    ps = ps_mm.tile([128, 512], f32, tag="mm")
                    for kc in range(2):
                        nc.tensor.matmul(ps[:], wqkvT[:, kc, lo:lo + 128],
                                         x_sb[:, kc, nt * 512:(nt + 1) * 512],
                                         start=(kc == 0), stop=(kc == 1))
                    nc.scalar.copy(dst[:, nt * 512:(nt + 1) * 512], ps[:])

            # ---- v transpose: vT[j128, (h,d)] per j-chunk ----
            for jc in range(NJC):
                pst = ps_t.tile([128, 512], f32, tag="pst")
                nc.tensor.transpose(pst[:, 0:128],
                                    v_sb[:, jc * 128:(jc + 1) * 128], ident[:])
                nc.vector.tensor_copy(vT_sb[:, jc, :], pst[:, 0:128])

            # ---- block-diag q stationary for all i-chunks ----
            qbd = big.tile([128, NIC, 128], f32, tag="qbd")
            nc.any.memset(qbd[:], 0.0)
            for h in range(H):
                nc.vector.tensor_copy(
                    qbd[h * 32:(h + 1) * 32, :, h * 32:(h + 1) * 32],
                    q_sb[h * 32:(h + 1) * 32, :].rearrange(
                        "p (ic w) -> p ic w", w=32),
                )

            hidden = big.tile([128, I], f32, tag="hidden")

            # ---- main attention loop ----
            # Per 32-row i-chunk: 4 j-tiles of [(h,i32)=128, 512] sim+bias
            # PSUM accumulation -> exp (row sums via accum_out) -> PE
            # transpose per 128-chunk -> DVE drain -> N=128 av matmuls
            # accumulating [(h,d), (h,i32)] over all 16 j-chunks.
            for ic in range(NIC):
                isl = slice(ic * 32, (ic + 1) * 32)
                rs4 = smallp.tile([128, 4], f32, tag="rs4", name="rs4")
                av = ps_av.tile([128, 128], f32, tag="av", name="av")
                for jt in range(NJT):
                    jsl = slice(jt * 512, (jt + 1) * 512)
                    indA = indp.tile([128, 512], f32, tag="indA", name="indA")
                    nc.sync.dma_start(indA[:], ind_d[ic, jt, 0:128, :])
                    indB = indp.tile([32, 512], f32, tag="indB", name="indB")
                    nc.sync.dma_start(indB[:], ind_d[ic, jt, 128:160, :])

                    ps = ps_mm.tile([128, 512], f32, tag="mm", name="ps")
                    nc.tensor.matmul(ps[:], qbd[:, ic, :], k_sb[:, jsl],
                                     start=True, stop=False)
                    nc.tensor.matmul(ps[:], s0[:], indA[:],
                                     start=False, stop=False)
                    nc.tensor.matmul(ps[:], s1[:], indB[:],
                                     start=False, stop=True)

                    e = ep.tile([128, 512], f32, tag="e", name="e")
                    nc.scalar.activation(e[:], ps[:], Act.Exp,
                                         accum_out=rs4[:, jt:jt + 1])

                    pst = ps_t.tile([128, 512], f32, tag="pst", name="pst")
                    for kc in range(4):
                        nc.tensor.transpose(pst[:, kc * 128:(kc + 1) * 128],
                                            e[:, kc * 128:(kc + 1) * 128],
                                            ident[:])
                    et = etp.tile([128, 512], f32, tag="et", name="et")
                    nc.vector.tensor_copy(et[:], pst[:])

                    for kc in range(4):
                        jc = jt * 4 + kc
                        nc.tensor.matmul(av[:], vT_sb[:, jc, :],
                                         et[:, kc * 128:(kc + 1) * 128],
                                         start=(jc == 0), stop=(jc == NJC - 1),
                                         skip_group_check=True)

                # row sums -> reciprocal -> free dim (DVE 32x32 transpose)
                rs1 = smallp.tile([128, 1], f32, tag="rs1", name="rs1")
                nc.vector.tensor_reduce(rs1[:], rs4[:],
                                        axis=mybir.AxisListType.X, op=Alu.add)
                recip32 = smallp.tile([128, 32], f32, tag="recip32",
                                      name="recip32")
                nc.vector.reciprocal(recip32[:], rs1[:].to_broadcast((128, 32)))
                rsT = smallp.tile([128, 32], f32, tag="rsT", name="rsT")
                nc.vector.transpose(rsT[:], recip32[:])
                # rsT[32h+y, i'] = 1/rowsum(h, i') for every y

                for h in range(H):
                    hsl = slice(h * 32, (h + 1) * 32)
                    nc.vector.scalar_tensor_tensor(
                        out=hidden[hsl, isl],
                        in0=av[hsl, hsl],
                        scalar=1.0,
                        in1=rsT[hsl, 0:32],
                        op0=Alu.mult,
                        op1=Alu.mult,
                    )

            # ---- output projection ----
            for oc in range(2):
                for it in range(2):
                    ps = ps_mm.tile([128, 512], f32, tag="mm")
                    nc.tensor.matmul(ps[:], woutT[:, oc, :],
                                     hidden[:, it * 512:(it + 1) * 512],
                                     start=True, stop=True)
                    osb = smallp.tile([128, 512], f32, tag="osb")
                    nc.scalar.add(osb[:], ps[:], bout[:, oc:oc + 1])
                    nc.sync.dma_start(
                        out_d[oc * 128:(oc + 1) * 128, it * 512:(it + 1) * 512],
                        osb[:])

    nc.compile()
    return nc


def _host_prep(w_qkv, w_ind, w_out, b_out):
    wqkv_s = np.ascontiguousarray(w_qkv, dtype=np.float32).copy()
    wqkv_s[:HID] *= np.float32(DH ** -0.5)
    wqkvT = np.ascontiguousarray(wqkv_s.T)          # (256, 384)
    wqkvT = np.ascontiguousarray(
        wqkvT.reshape(2, 128, 3 * HID))              # (2,128,384)
    wqkvT = np.ascontiguousarray(wqkvT.transpose(1, 0, 2))  # (128,2,384)

    S0 = np.zeros((128, 128), np.float32)
    S1 = np.zeros((32, 128), np.float32)
    ii = np.arange(32)
    for h in range(H):
        for c in range(4):
            S0[c * 32 + ii, h * 32 + ii] = w_ind[h, c]
        S1[ii, h * 32 + ii] = w_ind[h, 4]

    woutT = np.ascontiguousarray(w_out.T.astype(np.float32))     # (128, 256)
    woutT = np.ascontiguousarray(
        woutT.reshape(128, 2, 128))                  # (128,2,128)
    bout = np.ascontiguousarray(
        b_out.astype(np.float32).reshape(2, 128).T)  # (128,2)
    return wqkvT, S0, S1, woutT, bout


def _tile_ind(ind):
    """(C, I, N) -> (NIC, NJT, C*32, 512) with each tile contiguous.

    Row c*32+i of tile (ic, jt) = ind[c, ic*32+i, jt*512:(jt+1)*512], the
    exact (c, i32) partition layout the S0/S1 bias stationaries expect.
    """
    t = ind.reshape(C, NIC, 32, NJT, 512).transpose(1, 3, 0, 2, 4)
    return np.ascontiguousarray(t.reshape(NIC, NJT, C * 32, 512))


def kernel(x, indicator, w_qkv, w_ind, w_out, b_out):
    global _PROG
    from concourse.bass_utils import run_bass_kernel_spmd

    if _PROG is None:
        _PROG = _build_program()
    nc = _PROG

    x = np.ascontiguousarray(np.asarray(x, dtype=np.float32))
    indicator = np.asarray(indicator, dtype=np.float32)
    wqkvT, S0, S1, woutT, bout = _host_prep(
        np.asarray(w_qkv), np.asarray(w_ind), np.asarray(w_out),
        np.asarray(b_out))

    in_maps = []
    for core in range(NCORES):
        b, ih = core // 2, core % 2
        i0 = ih * I
        in_maps.append({
            "x": x[b],
            "xq": np.ascontiguousarray(x[b][:, i0:i0 + I]),
            "ind": _tile_ind(indicator[b, :, i0:i0 + I, :]),
            "wqkvT": wqkvT,
            "s0": S0,
            "s1": S1,
            "woutT": woutT,
            "bout": bout,
        })

    trace = os.environ.get("EXT_ATTN_TRACE") == "1"
    res = run_bass_kernel_spmd(nc, in_maps, list(range(NCORES)), trace=trace)
    global LAST_EXEC_NS, LAST_RESULTS
    LAST_EXEC_NS = res.exec_time_ns
    LAST_RESULTS = res
    out = np.empty((B, DIM, N), np.float32)
    for core in range(NCORES):
        b, ih = core // 2, core % 2
        out[b, :, ih * I:(ih + 1) * I] = res.results[core]["out"]
    return out


if __name__ == "__main__":
    rng = np.random.default_rng(0)
    ins = {
        "x": rng.standard_normal((B, DIM, N), dtype=np.float32),
        "indicator": rng.standard_normal((B, C, N, N), dtype=np.float32),
        "w_qkv": rng.standard_normal((3 * HID, DIM), dtype=np.float32) * DIM ** -0.5,
        "w_ind": rng.standard_normal((H, C), dtype=np.float32) * C ** -0.5,
        "w_out": rng.standard_normal((DIM, HID), dtype=np.float32) * HID ** -0.5,
        "b_out": np.zeros((DIM,), np.float32),
    }
    out = kernel(**ins)
    print("kernel ran, out shape", out.shape, "mean", float(np.abs(out).mean()))



# revision 32
# speedup vs baseline: 3.0402x; 3.0402x over previous
"""ExtAttention Trainium2 kernel (8 NeuronCores, SPMD).

Sharding: 8 cores = 4 batches x 2 query-row halves. Each core handles
batch b = core//2 and query rows [ih*1024, ih*1024+1024) with ih = core%2.
Softmax is over the key axis j (free dim), so row-sharding needs no
collectives; each core reads exactly its slice of the dominant `indicator`
tensor once (bf16 on the wire: 21 MB/core).

Per-core dataflow (n=2048 keys, I=1024 query rows, H=4 heads, DH=32),
all matmul operands bf16 (1 cyc/row on PE vs 4 for fp32), PSUM fp32:
  - qkv projection on PE (q only for the local row half; scale folded
    into w_q); drains on DVE.
  - indicator fetched with ONE big DMA per 2 i-chunks per region
    (A: channels 0-3 as [(c,i32)=128, 8x512], B: channel 4 repacked to
    [(jt,i32)=128, 2x512]) - 32 DMAs total instead of 256 (SWDGE
    generation costs ~1us per dma_start).
  - per 32-row i-chunk and 512-col j-tile: one PSUM tile [(h,i32), 512]
    accumulates sim (block-diag q stationary) + bias A (K=128) +
    bias B (K=32); ACT exp PSUM->SBUF bf16 with accum_out row sums.
  - software pipeline: transpose/av for tile t-2 are emitted after the
    sim/bias matmuls of tile t so PE never waits on ACT.
  - av: per-head matmuls lhsT=v^T[j,(h,d)] slice, rhs=E^T[j,(h,i32)]
    slice -> av[(h,d), i32] accumulated over all 16 j-chunks; the
    [(h,d), i32] layout lets ONE DVE STT apply the 1/rowsum scaling.
  - output projection (w_out^T stationary) + bias, DMA out (256, 1024).
"""

import os
import sys

import numpy as np

for _p in ("/opt/trn_rl_repo", "/root/.axon_site/_ro/trn_rl_repo"):
    if os.path.isdir(_p) and _p not in sys.path:
        sys.path.insert(0, _p)

B, DIM, N, C, H, DH = 4, 256, 2048, 5, 4, 32
HID = H * DH            # 128
NCORES = 8
I = N // 2              # 1024 query rows per core
NIC = I // 32           # 32 i-chunks
NJT = N // 512          # 4 j-tiles
NJC = N // 128          # 16 j-chunks
NP = NIC // 2           # 16 i-chunk pairs (DMA granularity)

_PROG = None
LAST_EXEC_NS = None
LAST_RESULTS = None


def _build_program(repeat=1):
    from contextlib import ExitStack

    import concourse.mybir as mybir
    import concourse.tile as tile
    from concourse import bacc
    from concourse.masks import make_identity

    f32 = mybir.dt.float32
    bf16 = mybir.dt.bfloat16
    Alu = mybir.AluOpType
    Act = mybir.ActivationFunctionType

    nc = bacc.Bacc("TRN2", target_bir_lowering=False, debug=False,
                   num_devices=NCORES)

    x_d = nc.dram_tensor("x", [DIM, N], bf16, kind="ExternalInput").ap()
    xq_d = nc.dram_tensor("xq", [DIM, I], bf16, kind="ExternalInput").ap()
    indA_d = nc.dram_tensor("indA", [NP, 128, 8, 512], bf16,
                            kind="ExternalInput").ap()
    indB_d = nc.dram_tensor("indB", [NP, 64, 2, 2, 512], bf16,
                            kind="ExternalInput").ap()
    wqkvT_d = nc.dram_tensor("wqkvT", [128, 2, 3 * HID], bf16,
                             kind="ExternalInput").ap()
    s0_d = nc.dram_tensor("s0", [128, 128], bf16, kind="ExternalInput").ap()
    s1_d = nc.dram_tensor("s1", [64, 128], bf16, kind="ExternalInput").ap()
    woutT_d = nc.dram_tensor("woutT", [128, 2, 128], bf16,
                             kind="ExternalInput").ap()
    bout_d = nc.dram_tensor("bout", [128, 2], f32, kind="ExternalInput").ap()
    out_d = nc.dram_tensor("out", [DIM, I], f32, kind="ExternalOutput").ap()

    with tile.TileContext(nc) as tc, ExitStack() as ctx:
        const = ctx.enter_context(tc.tile_pool(name="const", bufs=1))
        big = ctx.enter_context(tc.tile_pool(name="big", bufs=1))
        indp = ctx.enter_context(tc.tile_pool(name="indp", bufs=2))
        indbp = ctx.enter_context(tc.tile_pool(name="indbp", bufs=2))
        ep = ctx.enter_context(tc.tile_pool(name="ep", bufs=4))
        etp = ctx.enter_context(tc.tile_pool(name="etp", bufs=2))
        smallp = ctx.enter_context(tc.tile_pool(name="smallp", bufs=3))
        ps_mm = ctx.enter_context(tc.tile_pool(name="ps_mm", bufs=4, space="PSUM"))
        ps_t = ctx.enter_context(tc.tile_pool(name="ps_t", bufs=2, space="PSUM"))
        ps_av = ctx.enter_context(tc.tile_pool(name="ps_av", bufs=2, space="PSUM"))

        for _rep in range(repeat):
            # ---- constants ----
            wqkvT = const.tile([128, 2, 3 * HID], bf16, tag="wqkvT")
            nc.sync.dma_start(wqkvT[:], wqkvT_d)
            s0 = const.tile([128, 128], bf16, tag="s0")
            nc.sync.dma_start(s0[:], s0_d)
            s1 = const.tile([64, 128], bf16, tag="s1")
            nc.sync.dma_start(s1[:], s1_d)
            woutT = const.tile([128, 2, 128], bf16, tag="woutT")
            nc.sync.dma_start(woutT[:], woutT_d)
            bout = const.tile([128, 2], f32, tag="bout")
            nc.sync.dma_start(bout[:], bout_d)
            ident = const.tile([128, 128], bf16, tag="ident")
            make_identity(nc, ident[:])

            # ---- load x ----
            x_sb = big.tile([128, 2, N], bf16, tag="x_sb")
            xq_sb = big.tile([128, 2, I], bf16, tag="xq_sb")
            for kc in range(2):
                nc.sync.dma_start(x_sb[:, kc, :], x_d[kc * 128:(kc + 1) * 128, :])
                nc.sync.dma_start(xq_sb[:, kc, :], xq_d[kc * 128:(kc + 1) * 128, :])

            # ---- qkv projection (PE), drains on DVE ----
            q_sb = big.tile([128, I], bf16, tag="q_sb")    # [(h,d), i]; scale folded
            k_sb = big.tile([128, N], bf16, tag="k_sb")    # [(h,d), j]
            v_sb = big.tile([128, N], bf16, tag="v_sb")    # [(h,d), j]
            vT_sb = big.tile([128, NJC, 128], bf16, tag="vT_sb")  # [j128, jc, (h,d)]

            for nt in range(I // 512):
                ps = ps_mm.tile([128, 512], f32, tag="mm")
                for kc in range(2):
                    nc.tensor.matmul(ps[:], wqkvT[:, kc, 0:128],
                                     xq_sb[:, kc, nt * 512:(nt + 1) * 512],
                                     start=(kc == 0), stop=(kc == 1))
                nc.vector.tensor_copy(q_sb[:, nt * 512:(nt + 1) * 512], ps[:])
            for dst, lo in ((k_sb, 128), (v_sb, 256)):
                for nt in range(N // 512):
                    ps = ps_mm.tile([128, 512], f32, tag="mm")
                    for kc in range(2):
                        nc.tensor.matmul(ps[:], wqkvT[:, kc, lo:lo + 128],
                                         x_sb[:, kc, nt * 512:(nt + 1) * 512],
                                         start=(kc == 0), stop=(kc == 1))
                    nc.vector.tensor_copy(dst[:, nt * 512:(nt + 1) * 512], ps[:])

            # ---- v transpose: vT[j128, (h,d)] per j-chunk ----
            for jc in range(NJC):
                pst = ps_t.tile([128, 512], bf16, tag="pst")
                nc.tensor.transpose(pst[:, 0:128],
                                    v_sb[:, jc * 128:(jc + 1) * 128], ident[:])
                nc.vector.tensor_copy(vT_sb[:, jc, :], pst[:, 0:128])

            # ---- block-diag q stationary for all i-chunks ----
            qbd = big.tile([128, NIC, 128], bf16, tag="qbd")
            nc.any.memset(qbd[:], 0.0)
            for h in range(H):
                nc.vector.tensor_copy(
                    qbd[h * 32:(h + 1) * 32, :, h * 32:(h + 1) * 32],
                    q_sb[h * 32:(h + 1) * 32, :].rearrange(
                        "p (ic w) -> p ic w", w=32),
                )

            hidden = big.tile([128, I], bf16, tag="hidden")

            # ---- main attention loop, software-pipelined by 2 tiles ----
            TILES = NIC * NJT
            state = {}
            indA = indB = None
            rs4 = av = None
            rs4_of = {}
            av_of = {}

            for t in range(TILES + 2):
                if t < TILES:
                    ic, jt = divmod(t, NJT)
                    if jt == 0 and ic % 2 == 0:
                        p = ic // 2
                        indA = indp.tile([128, 8, 512], bf16, tag="indA",
                                         name="indA")
                        nc.sync.dma_start(indA[:], indA_d[p])
                        indB = indbp.tile([64, 2, 2, 512], bf16, tag="indB",
                                          name="indB")
                        nc.sync.dma_start(indB[:], indB_d[p])
                    if jt == 0:
                        rs4 = smallp.tile([128, 4], f32, tag="rs4", name="rs4")
                        av = ps_av.tile([128, 128], f32, tag="av", name="av")
                        rs4_of[ic] = rs4
                        av_of[ic] = av

                    icp = ic % 2
                    ps = ps_mm.tile([128, 512], f32, tag="mm", name="ps")
                    nc.tensor.matmul(ps[:], qbd[:, ic, :],
                                     k_sb[:, jt * 512:(jt + 1) * 512],
                                     start=True, stop=False)
                    nc.tensor.matmul(ps[:], s0[:], indA[:, icp * 4 + jt, :],
                                     start=False, stop=False)
                    jl = jt % 2
                    nc.tensor.matmul(ps[:], s1[jl * 32:(jl + 1) * 32, :],
                                     indB[jl * 32:(jl + 1) * 32, icp, jt // 2, :],
                                     start=False, stop=True)

                    e = ep.tile([128, 512], bf16, tag="e", name="e")
                    nc.scalar.activation(e[:], ps[:], Act.Exp,
                                         accum_out=rs4[:, jt:jt + 1])
                    state[t] = (ic, jt, e)

                if t >= 2:
                    sic, sjt, se = state.pop(t - 2)
                    sav = av_of[sic]
                    pst = ps_t.tile([128, 512], bf16, tag="pst", name="pst")
                    for kc in range(4):
                        nc.tensor.transpose(pst[:, kc * 128:(kc + 1) * 128],
                                            se[:, kc * 128:(kc + 1) * 128],
                                            ident[:])
                    et = etp.tile([128, 512], bf16, tag="et", name="et")
                    nc.vector.tensor_copy(et[:], pst[:])

                    for kc in range(4):
                        jc = sjt * 4 + kc
                        nc.tensor.matmul(
                            sav[:], vT_sb[:, jc, :],
                            et[:, kc * 128:(kc + 1) * 128],
                            start=(jc == 0), stop=(jc == NJC - 1),
                            skip_group_check=True)

                    if sjt == NJT - 1:
                        # epilogue for i-chunk sic: 1/rowsum in [(h,d), i32]
                        # orientation, single STT extraction.
                        srs4 = rs4_of.pop(sic)
                        av_of.pop(sic)
                        rs1 = smallp.tile([128, 1], f32, tag="rs1", name="rs1")
                        nc.vector.tensor_reduce(rs1[:], srs4[:],
                                                axis=mybir.AxisListType.X,
                                                op=Alu.add)
                        recip32 = smallp.tile([128, 32], f32, tag="recip32",
                                              name="recip32")
                        nc.vector.reciprocal(recip32[:],
                                             rs1[:].to_broadcast((128, 32)))
                        rsT = smallp.tile([128, 32], f32, tag="rsT",
                                          name="rsT")
                        nc.vector.transpose(rsT[:], recip32[:])
                        # rsT[32h+d, i'] = 1/rowsum(h, i') for every d
                        for h in range(H):
                            hsl = slice(h * 32, (h + 1) * 32)
                            nc.vector.scalar_tensor_tensor(
                                out=hidden[hsl, sic * 32:(sic + 1) * 32],
                                in0=sav[hsl, hsl],
                                scalar=1.0,
                                in1=rsT[hsl, :],
                                op0=Alu.mult,
                                op1=Alu.mult,
                            )

            # ---- output projection ----
            for oc in range(2):
                for it in range(2):
                    ps = ps_mm.tile([128, 512], f32, tag="mm")
                    nc.tensor.matmul(ps[:], woutT[:, oc, :],
                                     hidden[:, it * 512:(it + 1) * 512],
                                     start=True, stop=True)
                    osb = smallp.tile([128, 512], f32, tag="osb")
                    nc.scalar.add(osb[:], ps[:], bout[:, oc:oc + 1])
                    nc.sync.dma_start(
                        out_d[oc * 128:(oc + 1) * 128, it * 512:(it + 1) * 512],
                        osb[:])

    nc.compile()
    return nc


def _host_prep(w_qkv, w_ind, w_out, b_out):
    import ml_dtypes
    wqkv_s = np.ascontiguousarray(w_qkv, dtype=np.float32).copy()
    wqkv_s[:HID] *= np.float32(DH ** -0.5)
    wqkvT = np.ascontiguousarray(wqkv_s.T)          # (256, 384)
    wqkvT = np.ascontiguousarray(
        wqkvT.reshape(2, 128, 3 * HID))              # (2,128,384)
    wqkvT = np.ascontiguousarray(
        wqkvT.transpose(1, 0, 2)).astype(ml_dtypes.bfloat16)  # (128,2,384)

    S0 = np.zeros((128, 128), np.float32)
    S1 = np.zeros((64, 128), np.float32)
    ii = np.arange(32)
    for h in range(H):
        for c in range(4):
            S0[c * 32 + ii, h * 32 + ii] = w_ind[h, c]
        for jl in range(2):
            # replicated per 32-partition block so the lhsT slice shares
            # the rhs base partition (indB rows (jt%2)*32:...; matmul
            # base partitions must be in {0, 32, 64})
            S1[jl * 32 + ii, h * 32 + ii] = w_ind[h, 4]
    S0 = S0.astype(ml_dtypes.bfloat16)
    S1 = S1.astype(ml_dtypes.bfloat16)

    woutT = np.ascontiguousarray(w_out.T.astype(np.float32))     # (128, 256)
    woutT = np.ascontiguousarray(
        woutT.reshape(128, 2, 128)).astype(ml_dtypes.bfloat16)   # (128,2,128)
    bout = np.ascontiguousarray(
        b_out.astype(np.float32).reshape(2, 128).T)  # (128,2)
    return wqkvT, S0, S1, woutT, bout


def _tile_ind(ind):
    """(C, I, N) f32 -> (indA, indB) bf16 DMA superblocks.

    indA[p, c*32+i, icp*4+jt, :] = ind[c, (2p+icp)*32+i, jt*512:(jt+1)*512]
    for channels c in 0..3 -- the [(c,i32), 512] layout the S0 stationary
    expects, 8 j-tile blocks (2 i-chunks x 4 j-tiles) fetched per DMA.
    indB[p, (jt%2)*32+i, icp, jt//2, :] = ind[4, (2p+icp)*32+i, jt*512:...]:
    channel 4 repacked over 64 partitions (matmul base partitions are
    restricted to {0,32,64}); the S1 matmul for (ic, jt) streams rows
    (jt%2)*32:(jt%2+1)*32.
    """
    import ml_dtypes
    t = ind.reshape(C, NP, 2, 32, NJT, 512)
    A = t[0:4].transpose(1, 0, 3, 2, 4, 5).reshape(NP, 128, 8, 512)
    iB = t[4].reshape(NP, 2, 32, 2, 2, 512).transpose(
        0, 4, 2, 1, 3, 5).reshape(NP, 64, 2, 2, 512)
    return (np.ascontiguousarray(A).astype(ml_dtypes.bfloat16),
            np.ascontiguousarray(iB).astype(ml_dtypes.bfloat16))


def kernel(x, indicator, w_qkv, w_ind, w_out, b_out):
    global _PROG
    import ml_dtypes
    from concourse.bass_utils import run_bass_kernel_spmd

    if _PROG is None:
        _PROG = _build_program()
    nc = _PROG

    x = np.ascontiguousarray(np.asarray(x, dtype=np.float32))
    indicator = np.asarray(indicator, dtype=np.float32)
    wqkvT, S0, S1, woutT, bout = _host_prep(
        np.asarray(w_qkv), np.asarray(w_ind), np.asarray(w_out),
        np.asarray(b_out))

    in_maps = []
    for core in range(NCORES):
        b, ih = core // 2, core % 2
        i0 = ih * I
        iA, iB = _tile_ind(indicator[b, :, i0:i0 + I, :])
        in_maps.append({
            "x": x[b].astype(ml_dtypes.bfloat16),
            "xq": np.ascontiguousarray(
                x[b][:, i0:i0 + I]).astype(ml_dtypes.bfloat16),
            "indA": iA,
            "indB": iB,
            "wqkvT": wqkvT,
            "s0": S0,
            "s1": S1,
            "woutT": woutT,
            "bout": bout,
        })

    trace = os.environ.get("EXT_ATTN_TRACE") == "1"
    res = run_bass_kernel_spmd(nc, in_maps, list(range(NCORES)), trace=trace)
    global LAST_EXEC_NS, LAST_RESULTS
    LAST_EXEC_NS = res.exec_time_ns
    LAST_RESULTS = res
    out = np.empty((B, DIM, N), np.float32)
    for core in range(NCORES):
        b, ih = core // 2, core % 2
        out[b, :, ih * I:(ih + 1) * I] = res.results[core]["out"]
    return out


if __name__ == "__main__":
    rng = np.random.default_rng(0)
    ins = {
        "x": rng.standard_normal((B, DIM, N), dtype=np.float32),
        "indicator": rng.standard_normal((B, C, N, N), dtype=np.float32),
        "w_qkv": rng.standard_normal((3 * HID, DIM), dtype=np.float32) * DIM ** -0.5,
        "w_ind": rng.standard_normal((H, C), dtype=np.float32) * C ** -0.5,
        "w_out": rng.standard_normal((DIM, HID), dtype=np.float32) * HID ** -0.5,
        "b_out": np.zeros((DIM,), np.float32),
    }
    out = kernel(**ins)
    print("kernel ran, out shape", out.shape, "mean", float(np.abs(out).mean()))
